# revision 1
# baseline (speedup 1.0000x reference)
"""Trainium2 Bass kernel for nn_Net_28544352649361 (segment_reduce).

Reference computation:
    emb_tok[t]   = sum_d word_vectors[tokens[t], d]
    seg_sum[s]   = segment_sum(emb_tok, segment_ids)    (segment_ids sorted)
    lengths[s]   = segment counts
    sv[s]        = seg_sum[s] / max(lengths[s], 1)
    out[s, l]    = sv[s] * sum_d hidden_w[l, d] + hidden_b[l]
(the reference broadcasts the per-sentence scalar over d, so the final Linear
collapses to an outer product against hidden_w's row sums).

Distribution: data-parallel over sentences. Host cuts the token stream at
sentence boundaries s = 2048*c (8 binary searches), pads each shard to a fixed
135168 tokens, and runs one SPMD Bass program on all 8 NeuronCores. Each core:
  - per 128-token column: indirect-DMA row gather wv[tok], DVE reduce over d,
    indirect scatter-ADD (DMA CCE) of [emb, 1.0] into a DRAM accumulator
    indexed by in-shard segment id. 128 rotating accumulators keep concurrent
    scatter-adds race-free (columns 128 apart never share a segment).
  - combine accumulators, sv = sum/max(cnt,1), outer product with the
    broadcast row-sums of hidden_w, add bias, write [2048, 128] rows.
Host concatenates the 8 row blocks.
"""

import sys

sys.path.insert(0, "/opt/trn_rl_repo")

from contextlib import ExitStack

import numpy as np

import concourse.bass as bass
import concourse.tile as tile
from concourse import mybir
from concourse.bass import IndirectOffsetOnAxis
from concourse.vector_clock import ScopedClock

P = 128
F = 1056                 # token columns per core (128*1056 = 135168 slots)
SHARD = P * F
D = 128
NL = 128
NSENT = 16384
NCORES = 8
SENT_PER_CORE = NSENT // NCORES   # 2048
NBAG = 128               # rotating scatter-add accumulators
BAGROWS = 2176           # SENT_PER_CORE + junk rows for padding tokens
PAD_SEG = 2048           # in-shard segment id used for padding tokens (junk row)
UROWS = SENT_PER_CORE // P        # 16

_num_splits = [0]


# ---------------------------------------------------------------------------
# Workarounds for this walrus build (accepts only ONE sync-wait per
# instruction) and Tile's 8-lane DMA-sem round robin.
# ---------------------------------------------------------------------------
def _split_drain_and_barrier(self, tick_clock, wait_clock):
    nc = self.nc
    drain_inst = nc.sync.drain()
    wait_clock.add_sem_waits(
        drain_inst.ins, ScopedClock({None: tick_clock.global_clock})
    )
    mi = drain_inst.ins
    si = mi.sync_info
    if si is not None and si.on_wait is not None and len(si.on_wait) > 1:
        waits = list(si.on_wait)
        si.on_wait = waits[:1]
        for w in waits[1:]:
            extra = nc.sync.drain()
            emi = extra.ins
            esi = emi.sync_info
            if esi is None:
                emi.sync_info = mybir.SyncInfo(on_wait=[w], on_update=[])
            else:
                esi.on_wait = [w]
    nc.all_engine_barrier()
    assert self.sems is not None
    popped = nc._tile_sem_poison_stack.pop()
    assert popped is self._sem_poison
    nc.clear_and_free_semaphores(list(self.sems.allocated().values()))
    nc.all_engine_barrier()


def _apply_patches():
    if getattr(tile, "_segred_patched", False):
        return
    tile.TileContext._drain_and_barrier = _split_drain_and_barrier
    # NOTE: keep all 8 DMA sem lanes — _split_waits() below enforces the
    # 1-sync-wait-per-instruction compiler limit by hoisting surplus waits
    # onto NoOps, and a single lane would make every DMA consumer
    # transitively wait on all earlier DMAs' completions (full round-trip
    # serialization, ~9 us per column).
    tile._segred_patched = True


def _split_waits(nc):
    """Hoist surplus sync-waits onto same-engine NoOps placed just before the
    waiter; the engine sequencer executes them in order."""
    import bass_rust

    for f in nc.m.functions:
        for bb in f.blocks:
            new_list = []
            changed = False
            for inst in bb.instructions:
                si = inst.sync_info
                if si is not None and si.on_wait is not None and len(si.on_wait) > 1:
                    waits = list(si.on_wait)
                    si.on_wait = waits[-1:]
                    for w in waits[:-1]:
                        _num_splits[0] += 1
                        nop = bass_rust.InstNoOp(
                            name=f"WSPLIT-{_num_splits[0]}", ins=[], outs=[]
                        )
                        nop.engine = inst.engine
                        nop.sync_info = mybir.SyncInfo(on_wait=[w], on_update=[])
                        new_list.append(nop)
                    changed = True
                new_list.append(inst)
            if changed:
                bb.instructions = new_list


# ---------------------------------------------------------------------------
# Device program (identical for all cores; per-core data via in_maps)
# ---------------------------------------------------------------------------
def build_program():
    _apply_patches()
    nc = bass.Bass()
    f32 = mybir.dt.float32
    i32 = mybir.dt.int32

    wv = nc.declare_dram_parameter("wv", [100352, D], f32, isOutput=False)
    toks = nc.declare_dram_parameter("toks", [P, F], i32, isOutput=False)
    segs = nc.declare_dram_parameter("segs", [P, F], i32, isOutput=False)
    hwT = nc.declare_dram_parameter("hwT", [D, NL], f32, isOutput=False)
    hb = nc.declare_dram_parameter("hb", [1, NL], f32, isOutput=False)
    out = nc.declare_dram_parameter("out", [SENT_PER_CORE, NL], f32, isOutput=True)

    bags = [nc.dram_tensor(f"bag{r}", [BAGROWS, 2], f32) for r in range(NBAG)]

    with ExitStack() as ctx:
        tc = ctx.enter_context(tile.TileContext(nc))
        const = ctx.enter_context(tc.tile_pool(name="const", bufs=1))
        gp = ctx.enter_context(tc.tile_pool(name="g", bufs=6))
        small = ctx.enter_context(tc.tile_pool(name="small", bufs=2))
        psum = ctx.enter_context(tc.tile_pool(name="ps", bufs=2, space="PSUM"))

        tok_sb = const.tile([P, F], i32)
        seg_sb = const.tile([P, F], i32)
        nc.sync.dma_start(tok_sb[:], toks[:])
        nc.sync.dma_start(seg_sb[:], segs[:])

        # zero-init the accumulators
        z = const.tile([P, 2 * BAGROWS // P], f32)
        nc.vector.memset(z[:], 0.0)
        for r in range(NBAG):
            nc.sync.dma_start(bags[r][:], z[:])

        # main loop: gather rows -> reduce -> scatter-add [emb, 1] into bag.
        # Payload tiles rotate through a pool so the per-column chains pipeline
        # (a single shared payload array would serialize every chain through
        # tile-granular dependency tracking).
        sp = ctx.enter_context(tc.tile_pool(name="s2", bufs=12))
        for k in range(F):
            g = gp.tile([P, D], f32, tag="g")
            nc.gpsimd.indirect_dma_start(
                out=g[:],
                out_offset=None,
                in_=wv[:],
                in_offset=IndirectOffsetOnAxis(ap=tok_sb[:, k : k + 1], axis=0),
            )
            s2 = sp.tile([P, 2], f32, tag="s2")
            nc.vector.memset(s2[:, 1:2], 1.0)
            nc.vector.tensor_reduce(
                s2[:, 0:1],
                g[:],
                axis=mybir.AxisListType.X,
                op=mybir.AluOpType.add,
            )
            nc.gpsimd.indirect_dma_start(
                out=bags[k % NBAG][:],
                out_offset=IndirectOffsetOnAxis(ap=seg_sb[:, k : k + 1], axis=0),
                in_=s2[:],
                in_offset=None,
                compute_op=mybir.AluOpType.add,
            )

        # combine the 128 accumulators: acc[p, u, c] = sum_r bag_r[u*128+p, c]
        acc = const.tile([P, UROWS, 2], f32)
        nc.vector.memset(acc[:], 0.0)
        for r in range(NBAG):
            w = gp.tile([P, UROWS, 2], f32, tag="w")
            nc.sync.dma_start(
                w[:],
                bags[r][:].rearrange("(u p) c -> p u c", p=P)[0:P, 0:UROWS, :],
            )
            nc.vector.tensor_tensor(
                out=acc[:], in0=acc[:], in1=w[:], op=mybir.AluOpType.add
            )

        # sv = seg_sum / max(len, 1)
        lens = small.tile([P, UROWS], f32)
        nc.vector.tensor_scalar_max(lens[:], acc[:, :, 1], 1.0)
        rec = small.tile([P, UROWS], f32)
        nc.vector.reciprocal(rec[:], lens[:])
        sv = small.tile([P, UROWS], f32)
        nc.vector.tensor_tensor(
            out=sv[:], in0=acc[:, :, 0], in1=rec[:], op=mybir.AluOpType.mult
        )

        # broadcast row-sums of hidden_w and the bias to all partitions
        hwT_sb = const.tile([D, NL], f32)
        nc.sync.dma_start(hwT_sb[:], hwT[:])
        hb_sb = const.tile([1, NL], f32)
        nc.sync.dma_start(hb_sb[:], hb[:])
        ones_p = const.tile([P, 1], f32)
        nc.vector.memset(ones_p[:], 1.0)
        ones_1 = const.tile([1, P], f32)
        nc.vector.memset(ones_1[:], 1.0)

        wrow_ps = psum.tile([1, NL], f32, tag="ps1")
        nc.tensor.matmul(wrow_ps[:], ones_p[:], hwT_sb[:], start=True, stop=True)
        wrow = small.tile([1, NL], f32)
        nc.scalar.copy(wrow[:], wrow_ps[:])

        wb_ps = psum.tile([P, NL], f32, tag="ps2")
        nc.tensor.matmul(wb_ps[:], ones_1[:], wrow[:], start=True, stop=True)
        w_b = const.tile([P, NL], f32)
        nc.scalar.copy(w_b[:], wb_ps[:])

        bb_ps = psum.tile([P, NL], f32, tag="ps2")
        nc.tensor.matmul(bb_ps[:], ones_1[:], hb_sb[:], start=True, stop=True)
        b_b = const.tile([P, NL], f32)
        nc.scalar.copy(b_b[:], bb_ps[:])

        # out[u*128+p, l] = sv[p, u] * w_b[p, l] + b_b[p, l]
        out_sb = const.tile([P, UROWS, NL], f32)
        for u in range(UROWS):
            nc.vector.tensor_scalar(
                out=out_sb[:, u, :],
                in0=w_b[:],
                scalar1=sv[:, u : u + 1],
                scalar2=None,
                op0=mybir.AluOpType.mult,
            )
            nc.vector.tensor_tensor(
                out=out_sb[:, u, :],
                in0=out_sb[:, u, :],
                in1=b_b[:],
                op=mybir.AluOpType.add,
            )

        nc.sync.dma_start(
            out[:].rearrange("(u p) l -> p u l", p=P)[0:P, 0:UROWS, :], out_sb[:]
        )

    _split_waits(nc)
    return nc


_PROGRAM = None


def _get_program():
    global _PROGRAM
    if _PROGRAM is None:
        _PROGRAM = build_program()
    return _PROGRAM


def kernel(tokens, segment_ids, word_vectors, hidden_w, hidden_b):
    from concourse.bass_utils import run_bass_kernel_spmd

    tokens = np.asarray(tokens)
    segment_ids = np.asarray(segment_ids)
    word_vectors = np.asarray(word_vectors, dtype=np.float32)
    hidden_w = np.asarray(hidden_w, dtype=np.float32)
    hidden_b = np.asarray(hidden_b, dtype=np.float32)

    # replicate-pad the embedding table to the declared 100352 rows
    wv_pad = np.zeros((100352, D), dtype=np.float32)
    wv_pad[: word_vectors.shape[0]] = word_vectors
    hwT = np.ascontiguousarray(hidden_w.T)
    hb = hidden_b.reshape(1, NL)

    # sentence-aligned cuts: core c owns sentences [2048c, 2048(c+1))
    cuts = np.searchsorted(segment_ids, np.arange(NCORES + 1) * SENT_PER_CORE)
    in_maps = []
    for c in range(NCORES):
        lo, hi = int(cuts[c]), int(cuts[c + 1])
        n = hi - lo
        assert n <= SHARD, f"shard {c} has {n} tokens > {SHARD}"
        tk = np.zeros(SHARD, dtype=np.int32)
        sg = np.full(SHARD, PAD_SEG, dtype=np.int32)
        tk[:n] = tokens[lo:hi]
        sg[:n] = segment_ids[lo:hi] - c * SENT_PER_CORE
        in_maps.append(
            {
                "wv": wv_pad,
                "toks": tk.reshape(P, F),
                "segs": sg.reshape(P, F),
                "hwT": hwT,
                "hb": hb,
            }
        )

    nc = _get_program()
    res = run_bass_kernel_spmd(nc, in_maps, list(range(NCORES)))
    return np.concatenate([res.results[c]["out"] for c in range(NCORES)], axis=0)



# revision 17
# speedup vs baseline: 1.2973x; 1.2973x over previous
"""Trainium2 Bass kernel for nn_Net_28544352649361 (segment_reduce).

Reference computation:
    emb_tok[t]   = sum_d word_vectors[tokens[t], d]
    seg_sum[s]   = segment_sum(emb_tok, segment_ids)    (segment_ids sorted)
    lengths[s]   = segment counts
    sv[s]        = seg_sum[s] / max(lengths[s], 1)
    out[s, l]    = sv[s] * sum_d hidden_w[l, d] + hidden_b[l]

Key structural facts exploited here:
  * emb_tok[t] depends only on the ROW SUM of word_vectors at tokens[t], so
    the per-token gather can move 4 bytes instead of 512 once the row sums
    are precomputed (a single streaming pass over word_vectors).
  * segment_ids are sorted, so every segment is a contiguous token range and
    seg_sum[s] is a difference of two values of the token prefix-sum. The
    prefix-sum is computed with triangular-matrix matmuls on the PE array,
    eliminating the per-token DMA scatter-add of the previous version.

Distribution (8 cores):
  * Vocab rows are sharded 8 ways for the row-sum pass: each core reduces its
    12544 rows (DVE), writes the 50KB partial to DRAM, and one AllGather
    builds the full 100352-entry row-sum table on every core.
  * Tokens/sentences are sharded as before: core c owns sentences
    [2048c, 2048(c+1)); host cuts the sorted token stream at those boundaries
    and pads each shard to 147456 tokens (pad tokens index a zeroed table row).
  * Per core: one 4B-indirect gather per token block from the table,
    hierarchical prefix over the [128 x 1152] token grid
    (in-column prefix via L matmul + column offsets via two more triangular
    matmuls + PE transpose), prefix written to DRAM, two tiny boundary
    gathers, sv = diff/len, outer product with hidden_w row sums, bias add.
  * Host concatenates the 8 [2048, 128] row blocks.

Host-side prep is index arithmetic only (searchsorted over the sorted
segment_ids for shard cuts and segment boundary positions, vocab-id
remapping); all floating-point work happens on device.
"""

import sys

sys.path.insert(0, "/opt/trn_rl_repo")

from contextlib import ExitStack

import numpy as np

import concourse.bass as bass
import concourse.tile as tile
from concourse import mybir
from concourse.bass import IndirectOffsetOnAxis
from concourse.vector_clock import ScopedClock

P = 128
NB = 9                    # token column blocks in the prefix grid
F = NB * P                # 1152 grid columns (cols >= FG are zero padding)
FG = 1032                 # gathered token columns per core (132096 slots)
SHARD = P * FG            # padded token slots per core
D = 128
NL = 128
NSENT = 16384
NCORES = 8
SENT_PER_CORE = NSENT // NCORES   # 2048
U = SENT_PER_CORE // P            # 16
VOC_PAD = 100352                  # vocab rows padded to 8*12544
VSH = VOC_PAD // NCORES           # 12544 vocab rows per core
NCH = VSH // P                    # 98 row-chunks per core
WCH = 14                          # row-chunks per DVE reduce step (98 = 7*14)
PAD_TOK = VOC_PAD - 1             # padding token -> zeroed table row

_num_splits = [0]


# ---------------------------------------------------------------------------
# Workarounds for this walrus build (accepts only ONE sync-wait per
# instruction) and Tile's 8-lane DMA-sem round robin.
# ---------------------------------------------------------------------------
def _split_drain_and_barrier(self, tick_clock, wait_clock):
    nc = self.nc
    drain_inst = nc.sync.drain()
    wait_clock.add_sem_waits(
        drain_inst.ins, ScopedClock({None: tick_clock.global_clock})
    )
    mi = drain_inst.ins
    si = mi.sync_info
    if si is not None and si.on_wait is not None and len(si.on_wait) > 1:
        waits = list(si.on_wait)
        si.on_wait = waits[:1]
        for w in waits[1:]:
            extra = nc.sync.drain()
            emi = extra.ins
            esi = emi.sync_info
            if esi is None:
                emi.sync_info = mybir.SyncInfo(on_wait=[w], on_update=[])
            else:
                esi.on_wait = [w]
    nc.all_engine_barrier()
    assert self.sems is not None
    popped = nc._tile_sem_poison_stack.pop()
    assert popped is self._sem_poison
    nc.clear_and_free_semaphores(list(self.sems.allocated().values()))
    nc.all_engine_barrier()


def _apply_patches():
    if getattr(tile, "_segred_patched", False):
        return
    tile.TileContext._drain_and_barrier = _split_drain_and_barrier
    tile._segred_patched = True


def _split_waits(nc):
    """Hoist surplus sync-waits onto same-engine NoOps placed just before the
    waiter; the engine sequencer executes them in order."""
    import bass_rust

    for f in nc.m.functions:
        for bb in f.blocks:
            new_list = []
            changed = False
            for inst in bb.instructions:
                si = inst.sync_info
                if si is not None and si.on_wait is not None and len(si.on_wait) > 1:
                    waits = list(si.on_wait)
                    si.on_wait = waits[-1:]
                    for w in waits[:-1]:
                        _num_splits[0] += 1
                        nop = bass_rust.InstNoOp(
                            name=f"WSPLIT-{_num_splits[0]}", ins=[], outs=[]
                        )
                        nop.engine = inst.engine
                        nop.sync_info = mybir.SyncInfo(on_wait=[w], on_update=[])
                        new_list.append(nop)
                    changed = True
                new_list.append(inst)
            if changed:
                bb.instructions = new_list


# ---------------------------------------------------------------------------
# Device program (identical for all cores; per-core data via in_maps)
# ---------------------------------------------------------------------------
def build_program(split_waits=True, debug=False):
    _apply_patches()
    nc = bass.Bass(num_devices=NCORES)
    f32 = mybir.dt.float32
    i32 = mybir.dt.int32

    embd = gad = gbd = None
    if debug:
        embd = nc.declare_dram_parameter("embd", [P, F], f32, isOutput=True)
        gad = nc.declare_dram_parameter("gad", [P, U], f32, isOutput=True)
        gbd = nc.declare_dram_parameter("gbd", [P, U], f32, isOutput=True)

    wvs = nc.declare_dram_parameter("wvs", [P, NCH * P], f32, isOutput=False)
    toks = nc.declare_dram_parameter("toks", [P, FG], i32, isOutput=False)
    oa = nc.declare_dram_parameter("oa", [P, U], i32, isOutput=False)
    ob = nc.declare_dram_parameter("ob", [P, U], i32, isOutput=False)
    lns = nc.declare_dram_parameter("lns", [P, U], f32, isOutput=False)
    li = nc.declare_dram_parameter("li", [P, P], f32, isOutput=False)   # p <= i
    ls = nc.declare_dram_parameter("ls", [P, P], f32, isOutput=False)   # p < i
    l9 = nc.declare_dram_parameter("l9", [P, NB], f32, isOutput=False)  # 9x9 strict
    ident = nc.declare_dram_parameter("ident", [P, P], f32, isOutput=False)
    hwT = nc.declare_dram_parameter("hwT", [D, NL], f32, isOutput=False)
    hb = nc.declare_dram_parameter("hb", [1, NL], f32, isOutput=False)
    out = nc.declare_dram_parameter("out", [SENT_PER_CORE, NL], f32, isOutput=True)

    rs_shard = nc.dram_tensor("rs_shard", [VSH, 1], f32)
    table = nc.dram_tensor("rs_table", [VOC_PAD, 1], f32)
    pfx = nc.dram_tensor("pfx", [P * (F + 1), 1], f32)
    cpd = nc.dram_tensor("cpd", [NB * P, 1], f32)

    with ExitStack() as ctx:
        tc = ctx.enter_context(tile.TileContext(nc))
        const = ctx.enter_context(tc.tile_pool(name="const", bufs=1))
        wvp = ctx.enter_context(tc.tile_pool(name="wvp", bufs=2))
        embp = ctx.enter_context(tc.tile_pool(name="embp", bufs=3))
        small = ctx.enter_context(tc.tile_pool(name="small", bufs=2))
        pscs = ctx.enter_context(tc.tile_pool(name="pscs", bufs=1, space="PSUM"))
        psA = ctx.enter_context(tc.tile_pool(name="psA", bufs=2, space="PSUM"))
        psB = ctx.enter_context(tc.tile_pool(name="psB", bufs=2, space="PSUM"))
        psT = ctx.enter_context(tc.tile_pool(name="psT", bufs=2, space="PSUM"))

        # ---- small constant loads -----------------------------------------
        tok_sb = const.tile([P, FG], i32)
        nc.sync.dma_start(tok_sb[:], toks[:])
        oa_sb = const.tile([P, U], i32)
        nc.sync.dma_start(oa_sb[:], oa[:])
        ob_sb = const.tile([P, U], i32)
        nc.sync.dma_start(ob_sb[:], ob[:])
        lns_sb = const.tile([P, U], f32)
        nc.sync.dma_start(lns_sb[:], lns[:])
        li_sb = const.tile([P, P], f32)
        nc.sync.dma_start(li_sb[:], li[:])
        ls_sb = const.tile([P, P], f32)
        nc.sync.dma_start(ls_sb[:], ls[:])
        l9_sb = const.tile([P, NB], f32)
        nc.sync.dma_start(l9_sb[:], l9[:])
        id_sb = const.tile([P, P], f32)
        nc.sync.dma_start(id_sb[:], ident[:])
        hwT_sb = const.tile([D, NL], f32)
        nc.sync.dma_start(hwT_sb[:], hwT[:])
        hb_sb = const.tile([1, NL], f32)
        nc.sync.dma_start(hb_sb[:], hb[:])
        ones_p = li_sb[:, P - 1 : P]    # [128, 1] of ones (col 127 of p<=i)
        ones_1 = li_sb[0:1, :]          # [1, 128] of ones (row 0 of p<=i)

        # ---- phase A: vocab-shard row sums + AllGather --------------------
        rowsum = const.tile([P, NCH], f32)
        for ci in range(NCH // WCH):
            wt = wvp.tile([P, WCH, P], f32, tag="wv")
            nc.sync.dma_start(
                wt[:],
                wvs[:, ci * WCH * P : (ci + 1) * WCH * P].rearrange(
                    "p (w d) -> p w d", w=WCH
                ),
            )
            nc.vector.tensor_reduce(
                rowsum[:, ci * WCH : (ci + 1) * WCH],
                wt[:],
                axis=mybir.AxisListType.X,
                op=mybir.AluOpType.add,
            )
        nc.gpsimd.dma_start(
            rs_shard[:].rearrange("(p n) one -> p (n one)", p=P), rowsum[:]
        )
        nc.gpsimd.collective_compute(
            "AllGather",
            mybir.AluOpType.bypass,
            replica_groups=[list(range(NCORES))],
            ins=[rs_shard[:]],
            outs=[table[:]],
        )

        # ---- phase B: 4B/token gather + hierarchical prefix ---------------
        # token t = c*128 + p lives at [p, c]; one indirect DMA per column
        # (HW supports exactly 128 offsets per indirect DMA, one per
        # partition, each streaming one contiguous run -- here a single f32)
        emb = const.tile([P, F], f32)
        nc.vector.memset(emb[:, FG:F], 0.0)
        pp_sb = const.tile([P, NB, P], f32)     # in-column inclusive prefix
        cs_ps = pscs.tile([P, NB], f32, tag="cs")  # per-column sums (partition-major)
        for c in range(FG):
            nc.gpsimd.indirect_dma_start(
                out=emb[:, c : c + 1],
                out_offset=None,
                in_=table[:],
                in_offset=IndirectOffsetOnAxis(ap=tok_sb[:, c : c + 1], axis=0),
            )
        if debug:
            nc.sync.dma_start(embd[:], emb[:])
        for j in range(NB):
            blk = emb[:, j * P : (j + 1) * P]
            ppj = psA.tile([P, P], f32, tag="pp")
            nc.tensor.matmul(ppj[:], li_sb[:], blk, start=True, stop=True)
            nc.scalar.copy(pp_sb[:, j, :], ppj[:])
            nc.tensor.matmul(
                cs_ps[:, j : j + 1], blk, ones_p, start=True, stop=True
            )

        cs_sb = small.tile([P, NB], f32, tag="cs_sb")
        nc.scalar.copy(cs_sb[:], cs_ps[:])
        # block totals (partition-major over block index j)
        bt_full = psT.tile([P, P], f32, tag="t", name="bt_full")
        bt_ps = bt_full[0:NB, 0:1]
        nc.tensor.matmul(bt_ps, cs_sb[:], ones_p, start=True, stop=True)
        bt_sb = small.tile([NB, 1], f32, tag="bt_sb")
        nc.scalar.copy(bt_sb[:], bt_ps)
        # exclusive prefix over the 9 block totals
        bp_full = psT.tile([P, P], f32, tag="t", name="bp_full")
        bp_ps = bp_full[0:NB, 0:1]
        nc.tensor.matmul(bp_ps, l9_sb[0:NB, :], bt_sb[:], start=True, stop=True)
        bp_sb = small.tile([NB, 1], f32, tag="bp_sb")
        nc.scalar.copy(bp_sb[:], bp_ps)
        # within-block exclusive column prefix, then transpose to [j, q]
        cps_full = psT.tile([P, P], f32, tag="t", name="cps_full")
        cps_ps = cps_full[:, 0:NB]
        nc.tensor.matmul(cps_ps, ls_sb[:], cs_sb[:], start=True, stop=True)
        cps_sb = small.tile([P, NB], f32, tag="cps_sb")
        nc.scalar.copy(cps_sb[:], cps_ps)
        cpt_full = psT.tile([P, P], f32, tag="t", name="cpt_full")
        cpt_ps = cpt_full[0:NB, :]
        nc.tensor.transpose(cpt_ps, cps_sb[:], id_sb[:])
        cpt_sb = small.tile([NB, P], f32, tag="cpt_sb")
        nc.vector.tensor_scalar_add(cpt_sb[:], cpt_ps, bp_sb[:, 0:1])
        # flatten [9, 128] -> [1, 1152] through DRAM (partition-crossing
        # SBUF->SBUF DMA is not reliable on this build)
        nc.sync.dma_start(cpd[:].rearrange("(j q) one -> j (q one)", j=NB), cpt_sb[:])
        cpf_sb = small.tile([1, NB * P], f32, tag="cpf")
        nc.sync.dma_start(
            cpf_sb[:], cpd[:].rearrange("(one j) q -> one (j q)", one=1)
        )

        # pfx[p, k] = pp[p, k] + colprefix_excl[k]; column F is the zero slot
        pfx_sb = const.tile([P, F + 1], f32)
        nc.vector.memset(pfx_sb[:, F : F + 1], 0.0)
        CH = 3 * P  # 384 columns per broadcast matmul (moving dim <= 512)
        for ci in range(NB // 3):
            bb = psB.tile([P, CH], f32, tag="bb")
            nc.tensor.matmul(
                bb[:], ones_1, cpf_sb[:, ci * CH : (ci + 1) * CH],
                start=True, stop=True,
            )
            nc.vector.tensor_tensor(
                out=pfx_sb[:, ci * CH : (ci + 1) * CH],
                in0=pp_sb[:].rearrange("p j q -> p (j q)")[
                    :, ci * CH : (ci + 1) * CH
                ],
                in1=bb[:],
                op=mybir.AluOpType.add,
            )
        nc.sync.dma_start(
            pfx[:].rearrange("(p k) one -> p (k one)", p=P), pfx_sb[:]
        )

        # ---- phase C: boundary gathers, sv, outer product -----------------
        ga = small.tile([P, U], f32, tag="ga")
        gb = small.tile([P, U], f32, tag="gb")
        for u in range(U):
            nc.gpsimd.indirect_dma_start(
                out=ga[:, u : u + 1],
                out_offset=None,
                in_=pfx[:],
                in_offset=IndirectOffsetOnAxis(ap=oa_sb[:, u : u + 1], axis=0),
            )
            nc.gpsimd.indirect_dma_start(
                out=gb[:, u : u + 1],
                out_offset=None,
                in_=pfx[:],
                in_offset=IndirectOffsetOnAxis(ap=ob_sb[:, u : u + 1], axis=0),
            )
        if debug:
            nc.sync.dma_start(gad[:], ga[:])
            nc.sync.dma_start(gbd[:], gb[:])
        segsum = small.tile([P, U], f32, tag="ss")
        nc.vector.tensor_tensor(
            out=segsum[:], in0=ga[:], in1=gb[:], op=mybir.AluOpType.subtract
        )
        lmax = small.tile([P, U], f32, tag="lm")
        nc.vector.tensor_scalar_max(lmax[:], lns_sb[:], 1.0)
        rec = small.tile([P, U], f32, tag="rc")
        nc.vector.reciprocal(rec[:], lmax[:])
        sv = small.tile([P, U], f32, tag="sv")
        nc.vector.tensor_tensor(
            out=sv[:], in0=segsum[:], in1=rec[:], op=mybir.AluOpType.mult
        )

        # broadcast row-sums of hidden_w and the bias to all partitions
        wrow_full = psT.tile([P, P], f32, tag="t", name="wrow_full")
        wrow_ps = wrow_full[0:1, :]
        nc.tensor.matmul(wrow_ps, ones_p, hwT_sb[:], start=True, stop=True)
        wrow = small.tile([1, NL], f32, tag="wrow")
        nc.scalar.copy(wrow[:], wrow_ps)
        wb_ps = psT.tile([P, P], f32, tag="t")
        nc.tensor.matmul(wb_ps[:], ones_1, wrow[:], start=True, stop=True)
        w_b = const.tile([P, NL], f32)
        nc.scalar.copy(w_b[:], wb_ps[:])
        bb_ps = psT.tile([P, P], f32, tag="t")
        nc.tensor.matmul(bb_ps[:], ones_1, hb_sb[:], start=True, stop=True)
        b_b = const.tile([P, NL], f32)
        nc.scalar.copy(b_b[:], bb_ps[:])

        # out[u*128+p, l] = sv[p, u] * w_b[p, l] + b_b[p, l]
        out_sb = const.tile([P, U, NL], f32)
        for u in range(U):
            nc.vector.tensor_scalar(
                out=out_sb[:, u, :],
                in0=w_b[:],
                scalar1=sv[:, u : u + 1],
                scalar2=None,
                op0=mybir.AluOpType.mult,
            )
            nc.vector.tensor_tensor(
                out=out_sb[:, u, :],
                in0=out_sb[:, u, :],
                in1=b_b[:],
                op=mybir.AluOpType.add,
            )
        nc.sync.dma_start(
            out[:].rearrange("(u p) l -> p u l", p=P)[0:P, 0:U, :], out_sb[:]
        )

    if split_waits:
        _split_waits(nc)
    return nc


_PROGRAM = None


def _get_program():
    global _PROGRAM
    if _PROGRAM is None:
        _PROGRAM = build_program()
    return _PROGRAM


def kernel(tokens, segment_ids, word_vectors, hidden_w, hidden_b):
    from concourse.bass_utils import run_bass_kernel_spmd

    tokens = np.asarray(tokens)
    segment_ids = np.asarray(segment_ids)
    word_vectors = np.asarray(word_vectors, dtype=np.float32)
    hidden_w = np.asarray(hidden_w, dtype=np.float32)
    hidden_b = np.asarray(hidden_b, dtype=np.float32)

    voc = word_vectors.shape[0]
    wv_pad = np.zeros((VOC_PAD, D), dtype=np.float32)
    wv_pad[:voc] = word_vectors
    hwT = np.ascontiguousarray(hidden_w.T)
    hbr = hidden_b.reshape(1, NL)

    # vocab id -> row-sum table position (row r of shard c lands at
    # c*VSH + (r%128)*NCH + r//128 after the on-device [p, n] layout)
    def vmap(v):
        return (v // VSH) * VSH + (v % P) * NCH + (v % VSH) // P

    # constant matrices for the triangular-matmul prefix
    li_m = np.triu(np.ones((P, P), dtype=np.float32))          # p <= i
    ls_m = np.triu(np.ones((P, P), dtype=np.float32), k=1)     # p < i
    l9_m = np.zeros((P, NB), dtype=np.float32)
    l9_m[:NB, :] = np.triu(np.ones((NB, NB), dtype=np.float32), k=1)
    id_m = np.eye(P, dtype=np.float32)

    # sentence-aligned cuts + per-sentence token counts (index arithmetic)
    cuts = np.searchsorted(segment_ids, np.arange(NCORES + 1) * SENT_PER_CORE)
    ecnt = np.searchsorted(segment_ids, np.arange(NSENT + 1))

    in_maps = []
    for c in range(NCORES):
        lo, hi = int(cuts[c]), int(cuts[c + 1])
        n = hi - lo
        assert n <= SHARD, f"shard {c} has {n} tokens > {SHARD}"
        tk = np.full(SHARD, PAD_TOK, dtype=np.int64)
        tk[:n] = tokens[lo:hi]
        tkm = vmap(tk).astype(np.int32)
        toks_2d = np.ascontiguousarray(tkm.reshape(FG, P).T)

        ea = ecnt[c * SENT_PER_CORE + 1 : (c + 1) * SENT_PER_CORE + 1] - lo
        eb = ecnt[c * SENT_PER_CORE : (c + 1) * SENT_PER_CORE] - lo

        def posmap(e):
            t = e - 1
            pos = (t % P) * (F + 1) + t // P
            return np.where(e == 0, F, pos).astype(np.int32)

        oa_2d = np.ascontiguousarray(posmap(ea).reshape(U, P).T)
        ob_2d = np.ascontiguousarray(posmap(eb).reshape(U, P).T)
        ln_2d = np.ascontiguousarray(
            (ea - eb).astype(np.float32).reshape(U, P).T
        )

        shard = wv_pad[c * VSH : (c + 1) * VSH]
        wvs_2d = np.ascontiguousarray(
            shard.reshape(NCH, P, D).transpose(1, 0, 2).reshape(P, NCH * D)
        )

        in_maps.append(
            {
                "wvs": wvs_2d,
                "toks": toks_2d,
                "oa": oa_2d,
                "ob": ob_2d,
                "lns": ln_2d,
                "li": li_m,
                "ls": ls_m,
                "l9": l9_m,
                "ident": id_m,
                "hwT": hwT,
                "hb": hbr,
            }
        )

    nc = _get_program()
    res = run_bass_kernel_spmd(nc, in_maps, list(range(NCORES)))
    return np.concatenate([res.results[c]["out"] for c in range(NCORES)], axis=0)
